# revision 14
# baseline (speedup 1.0000x reference)
"""Trainium2 Bass kernel for nn_AdapterWithHyperNet (dense_mlp).

Computation (per reference.py):
    t  = x @ Rd^T                 [B,K]   (Rd = down_proj_right [K,D])
    t2 = batched H_b @ t          [B,K]   (hypernet_output [B,K,K])
    z  = sigmoid(t2 @ Ld^T + b1)  [B,BOT] (Ld = down_proj_left [BOT,K])
    s  = z @ Ru^T                 [B,K]   (Ru = up_proj_right [K,BOT])
    s2 = batched H_b @ s          [B,K]
    y  = s2 @ Lu^T (+ b2)         [B,D]   (Lu = up_proj_left [D,K])
    out = x + gamma * (y+b2 - mean_b(y+b2)) / sqrt(var_b + eps)

Distribution: pure data-parallel over the batch, B=8192 -> 1024 rows/core on
8 NeuronCores.  Batchnorm statistics are computed WITHOUT materializing y:
mean/var per feature d derive from G = sum_b s2_b s2_b^T [K,K] and
ssum = sum_b s2_b [K] via
    sum_b y[b,d]   = Lu[d,:] @ ssum
    sum_b y[b,d]^2 = Lu[d,:] @ G @ Lu[d,:]^T
so the cross-device all-reduce is a single tiny [65,64] tensor.  bias2
cancels exactly in the final expression (batchnorm is shift invariant), and
the normalization folds into a rescaled Lu plus a rank-1 shift row, so the
second pass is one matmul + residual add per tile.
"""

import os
import sys

for _p in ("/opt/trn_rl_repo", "/root/.axon_site/_ro/trn_rl_repo"):
    if os.path.isdir(_p) and _p not in sys.path:
        sys.path.append(_p)

import numpy as np

import concourse.bass as bass
import concourse.tile as tile
from concourse import mybir
from concourse.masks import make_identity
from concourse.vector_clock import ScopedClock
from concourse.bass_utils import run_bass_kernel_spmd

B, D, BOT, K = 8192, 4096, 1024, 64
EPS = 1e-5
N_CORES = 8
BC = B // N_CORES          # rows per core = 1024
CH = 128                   # rows per chunk
NCH = BC // CH             # chunks per core = 8
ND = D // 128              # 32 d-blocks of 128
NB = BOT // 128            # 8 bot-blocks of 128
NS = D // 512              # 8 output column slices of 512
F32 = mybir.dt.float32
AX = mybir.AxisListType
ALU = mybir.AluOpType
AF = mybir.ActivationFunctionType

# ---------------------------------------------------------------------------
# Walrus in this container rejects instructions with >1 semaphore waits; the
# TileContext final drain aggregates all outstanding waits onto one Drain.
# Split them across consecutive drains.
_MAX_WAITS = 1


def _patched_drain_and_barrier(self, tick_clock, wait_clock):
    drain_inst = self.nc.sync.drain()
    wait_clock.add_sem_waits(
        drain_inst.ins, ScopedClock({None: tick_clock.global_clock})
    )
    si = drain_inst.ins.sync_info
    waits = list(si.on_wait) if si is not None and si.on_wait else []
    if len(waits) > _MAX_WAITS:
        drain_inst.ins.sync_info = mybir.SyncInfo(
            on_wait=waits[:_MAX_WAITS], on_update=list(si.on_update or [])
        )
        rest = waits[_MAX_WAITS:]
        while rest:
            chunk, rest = rest[:_MAX_WAITS], rest[_MAX_WAITS:]
            d2 = self.nc.sync.drain()
            d2.ins.sync_info = mybir.SyncInfo(on_wait=chunk, on_update=[])
    self.nc.all_engine_barrier()
    popped = self.nc._tile_sem_poison_stack.pop()
    assert popped is self._sem_poison
    self.nc.clear_and_free_semaphores(list(self.sems.allocated().values()))
    self.nc.all_engine_barrier()


tile.TileContext._drain_and_barrier = _patched_drain_and_barrier


def _split_sync_waits(nc):
    """Walrus here allows at most one semaphore wait per instruction; move
    extra waits onto NoOps inserted just before, on the same engine."""
    for fn in nc.m.functions:
        for blk in fn.blocks:
            changed = False
            out = []
            for inst in blk.instructions:
                si = inst.sync_info
                waits = list(si.on_wait) if si is not None and si.on_wait else []
                if len(waits) > _MAX_WAITS:
                    changed = True
                    extra = waits[:-_MAX_WAITS]
                    for k, w in enumerate(extra):
                        nop = mybir.InstNoOp(
                            name=f"{inst.name}_ws{k}", ins=[], outs=[]
                        )
                        nop.engine = inst.engine
                        nop.sync_info = mybir.SyncInfo(on_wait=[w], on_update=[])
                        out.append(nop)
                    inst.sync_info = mybir.SyncInfo(
                        on_wait=waits[-_MAX_WAITS:],
                        on_update=list(si.on_update or []),
                    )
                out.append(inst)
            if changed:
                blk.instructions = out
# ---------------------------------------------------------------------------


def _install_ntff_hook_shim():
    """The agent image's antenv lacks axon_hooks; provide it so
    run_bass_kernel_spmd(trace=True) can capture NTFF profiles via the
    axon PJRT sidechannel."""
    try:
        from antenv.axon_hooks import get_axon_ntff_profile_hook  # noqa: F401
        return
    except ImportError:
        pass
    import types
    import ctypes
    import contextlib

    _hook = None
    so_path = "/opt/axon/libaxon_pjrt.so"
    if os.path.exists(so_path):
        lib = ctypes.CDLL(so_path)
        if hasattr(lib, "axon_start_nrt_profile"):
            lib.axon_start_nrt_profile.argtypes = [
                ctypes.POINTER(ctypes.c_int64), ctypes.c_size_t
            ]
            lib.axon_start_nrt_profile.restype = ctypes.c_int64
            lib.axon_stop_nrt_profile.argtypes = [ctypes.c_char_p]
            lib.axon_stop_nrt_profile.restype = ctypes.c_int64

            @contextlib.contextmanager
            def _h(output_dir, device_ids):
                import jax
                jax.devices()
                if device_ids:
                    ids = (ctypes.c_int64 * len(device_ids))(*device_ids)
                    rc = lib.axon_start_nrt_profile(ids, len(device_ids))
                else:
                    rc = lib.axon_start_nrt_profile(None, 0)
                if rc != 0:
                    raise RuntimeError(f"axon_start_nrt_profile rc={rc}")
                try:
                    yield
                finally:
                    n = lib.axon_stop_nrt_profile(str(output_dir).encode())
                    print(f"ntff profile: {n} file(s) -> {output_dir}",
                          file=sys.stderr)

            _hook = _h

    mod = types.ModuleType("antenv.axon_hooks")
    mod.get_axon_ntff_profile_hook = lambda: _hook
    mod.set_axon_ntff_profile_hook = lambda h: None
    sys.modules["antenv.axon_hooks"] = mod


def _patch_upload_artifacts():
    """No cloud egress in this container — keep profile artifacts local."""
    from concourse import bass_utils as _bu
    _bu.upload_artifacts = lambda tmpdir: str(tmpdir)


def build_nc():
    nc = bass.Bass(num_devices=N_CORES)

    x_in = nc.declare_dram_parameter("x", [BC, D], F32, isOutput=False)
    hyp_in = nc.declare_dram_parameter("hyp", [BC, K * K], F32, isOutput=False)
    rdt_in = nc.declare_dram_parameter("rdt", [D, K], F32, isOutput=False)
    ldt_in = nc.declare_dram_parameter("ldt", [K, BOT], F32, isOutput=False)
    rut_in = nc.declare_dram_parameter("rut", [BOT, K], F32, isOutput=False)
    lut_in = nc.declare_dram_parameter("lut", [K, D], F32, isOutput=False)
    gam_in = nc.declare_dram_parameter("gamma", [1, D], F32, isOutput=False)
    b1_in = nc.declare_dram_parameter("bias1", [BOT], F32, isOutput=False)
    out_ext = nc.declare_dram_parameter("out", [BC, D], F32, isOutput=True)

    g_bounce_in = nc.dram_tensor("g_bounce_in", [K + 1, K], F32)
    g_bounce_out = nc.dram_tensor("g_bounce_out", [K + 1, K], F32, addr_space="Shared")

    with tile.TileContext(nc, num_cores=N_CORES) as tc:
        from contextlib import ExitStack

        ctx = ExitStack()
        with ctx:
            const = ctx.enter_context(tc.tile_pool(name="const", bufs=1))
            g_psp = ctx.enter_context(tc.tile_pool(name="g_psp", bufs=1, space="PSUM"))

            ident = const.tile([128, 128], F32)
            make_identity(nc, ident[:])
            ones64 = const.tile([64, 64], F32)
            nc.gpsimd.memset(ones64[:], 1.0)

            rdt_sb = const.tile([128, ND, K], F32)   # (p,c,k) = RdT[c*128+p, k]
            nc.sync.dma_start(rdt_sb[:], rdt_in[:].rearrange("(c p) k -> p c k", p=128))
            ldt_sb = const.tile([K, BOT], F32)
            nc.sync.dma_start(ldt_sb[:], ldt_in[:])
            rut_sb = const.tile([128, NB, K], F32)
            nc.sync.dma_start(rut_sb[:], rut_in[:].rearrange("(c p) k -> p c k", p=128))
            lut_sb = const.tile([K, D], F32)
            nc.sync.dma_start(lut_sb[:], lut_in[:])
            gam_sb = const.tile([1, D], F32)
            nc.sync.dma_start(gam_sb[:], gam_in[:])
            b1_sb = const.tile([128, NB], F32)
            nc.sync.dma_start(b1_sb[:], b1_in[:].rearrange("(c p) -> p c", p=128))

            # s2 for all chunks, augmented with a ones column per chunk
            s2aug = const.tile([128, NCH * (K + 1)], F32)
            nc.gpsimd.memset(s2aug[:], 1.0)

            lut_aug = const.tile([K + 1, D], F32)    # [scale*LuT ; shift-row]
            g_acc = const.tile([K + 1, K], F32)      # accumulated G across chunks
            nc.gpsimd.memset(g_acc[:], 0.0)

            # ---------------- pass 1 -----------------
            with (
                tc.tile_pool(name="xp", bufs=2) as xp,
                tc.tile_pool(name="hp", bufs=2) as hp,
                tc.tile_pool(name="xtp", bufs=2) as xtp,
                tc.tile_pool(name="up", bufs=2) as up,
                tc.tile_pool(name="zp", bufs=2) as zp,
                tc.tile_pool(name="smp", bufs=3) as smp,
                tc.tile_pool(name="tps", bufs=2, space="PSUM") as tps,
                tc.tile_pool(name="sps", bufs=4, space="PSUM") as sps,
            ):
                for i in range(NCH):
                    rows = bass.ts(i, CH)
                    x_i = xp.tile([128, D], F32)
                    nc.sync.dma_start(x_i[:], x_in[rows, :])
                    hyp_i = hp.tile([128, K * K], F32)
                    nc.sync.dma_start(hyp_i[:], hyp_in[rows, :])

                    # transpose x chunk: 32 blocks of [128,128]
                    xt_i = xtp.tile([128, ND, 128], F32)
                    for g in range(ND // 4):
                        pst = tps.tile([128, 512], F32, tag="pst")
                        for j in range(4):
                            c = g * 4 + j
                            nc.tensor.transpose(
                                pst[:, bass.ts(j, 128)], x_i[:, bass.ts(c, 128)], ident[:]
                            )
                        nc.scalar.copy(
                            xt_i[:, g * 4:(g + 1) * 4, :].rearrange("p a b -> p (a b)"),
                            pst[:],
                        )

                    # t^T = RdT^T @ x^T : [64,128]
                    t_ps = sps.tile([K, 128], F32, tag="small")
                    for c in range(ND):
                        nc.tensor.matmul(
                            t_ps[:], rdt_sb[:, c, :], xt_i[:, c, :],
                            start=(c == 0), stop=(c == ND - 1),
                        )
                    tT_sb = smp.tile([K, 128], F32, tag="tT")
                    nc.scalar.copy(tT_sb[:], t_ps[:])
                    # t [128,64]
                    t_ps2 = sps.tile([128, K], F32, tag="small")
                    nc.tensor.transpose(t_ps2[:], tT_sb[:], ident[0:K, 0:K])
                    t_sb = smp.tile([128, K], F32, tag="t")
                    nc.scalar.copy(t_sb[:], t_ps2[:])

                    # hypernet 1: t2[b,k] = sum_j H[b,k,j] t[b,j]
                    u1 = up.tile([128, K, K], F32, tag="u")
                    nc.vector.tensor_tensor(
                        u1[:],
                        hyp_i[:].rearrange("p (a b) -> p a b", b=K),
                        t_sb[:].unsqueeze(1).broadcast_to([128, K, K]),
                        op=ALU.mult,
                    )
                    t2_sb = smp.tile([128, K], F32, tag="t")
                    nc.vector.tensor_reduce(t2_sb[:], u1[:], axis=AX.X, op=ALU.add)

                    # t2^T [64,128]
                    t2_ps = sps.tile([K, 128], F32, tag="small")
                    nc.tensor.transpose(t2_ps[:], t2_sb[:], ident[:])
                    t2T_sb = smp.tile([K, 128], F32, tag="tT")
                    nc.scalar.copy(t2T_sb[:], t2_ps[:])

                    # z^T chunks: sigmoid(LdT^T @ t2T + b1)
                    z_sb = zp.tile([128, NB, 128], F32)
                    for c in range(NB):
                        z_ps = sps.tile([128, 128], F32, tag="small")
                        nc.tensor.matmul(
                            z_ps[:], ldt_sb[:, bass.ts(c, 128)], t2T_sb[:],
                            start=True, stop=True,
                        )
                        nc.scalar.activation(
                            z_sb[:, c, :], z_ps[:], AF.Sigmoid, bias=b1_sb[:, c:c + 1]
                        )

                    # s^T = sum_c RuT_c^T @ zT_c : [64,128]
                    s_ps = sps.tile([K, 128], F32, tag="small")
                    for c in range(NB):
                        nc.tensor.matmul(
                            s_ps[:], rut_sb[:, c, :], z_sb[:, c, :],
                            start=(c == 0), stop=(c == NB - 1),
                        )
                    sT_sb = smp.tile([K, 128], F32, tag="tT")
                    nc.scalar.copy(sT_sb[:], s_ps[:])
                    s_ps2 = sps.tile([128, K], F32, tag="small")
                    nc.tensor.transpose(s_ps2[:], sT_sb[:], ident[0:K, 0:K])
                    s_sb = smp.tile([128, K], F32, tag="t")
                    nc.scalar.copy(s_sb[:], s_ps2[:])

                    # hypernet 2 -> s2 (straight into s2aug slot)
                    u2 = up.tile([128, K, K], F32, tag="u")
                    nc.vector.tensor_tensor(
                        u2[:],
                        hyp_i[:].rearrange("p (a b) -> p a b", b=K),
                        s_sb[:].unsqueeze(1).broadcast_to([128, K, K]),
                        op=ALU.mult,
                    )
                    s2_slot = s2aug[:, i * (K + 1): i * (K + 1) + K]
                    nc.vector.tensor_reduce(s2_slot, u2[:], axis=AX.X, op=ALU.add)

                    # G for this chunk: [65,64] = s2aug_i^T @ s2_i, accumulate in SBUF
                    g_ps = g_psp.tile([K + 1, K], F32, tag="g")
                    nc.tensor.matmul(
                        g_ps[:],
                        s2aug[:, i * (K + 1):(i + 1) * (K + 1)],
                        s2_slot,
                        start=True, stop=True,
                    )
                    nc.vector.tensor_tensor(g_acc[:], g_acc[:], g_ps[:], op=ALU.add)

            # ---------------- all-reduce + stats -----------------
            with (
                tc.tile_pool(name="stp", bufs=2) as stp,
                tc.tile_pool(name="stps", bufs=6, space="PSUM") as stps,
            ):
                nc.sync.dma_start(g_bounce_in[:], g_acc[:])
                nc.gpsimd.collective_compute(
                    "AllReduce", ALU.add,
                    replica_groups=[list(range(N_CORES))],
                    ins=[g_bounce_in[:]], outs=[g_bounce_out[:]],
                )
                gall = stp.tile([K + 1, K], F32, tag="g")
                nc.sync.dma_start(gall[:], g_bounce_out[:])

                # ssum replicated across columns: [64,64]
                ssum_row = stp.tile([1, K], F32, tag="ssrow", bufs=1)
                nc.scalar.copy(ssum_row[:], gall[K:K + 1, :])
                eps_col = stp.tile([K, 1], F32, tag="eps", bufs=1)
                nc.gpsimd.memset(eps_col[:], EPS)
                ssr_ps = stps.tile([K, K], F32, tag="ssr_ps", bufs=1)
                nc.tensor.matmul(ssr_ps[:], ssum_row[:], ones64[0:1, :],
                                 start=True, stop=True)
                ssr_sb = stp.tile([K, K], F32, tag="ssr", bufs=1)
                nc.scalar.copy(ssr_sb[:], ssr_ps[:])

                for sl in range(NS):
                    cols = bass.ts(sl, 512)
                    m1_ps = stps.tile([K, 512], F32, tag="m1", bufs=1)
                    nc.tensor.matmul(m1_ps[:], gall[0:K, :], lut_sb[:, cols],
                                     start=True, stop=True)
                    p_sb = stp.tile([K, 512], F32, tag="p", bufs=2)
                    nc.vector.tensor_tensor(p_sb[:], m1_ps[:], lut_sb[:, cols], op=ALU.mult)
                    q_ps = stps.tile([K, 512], F32, tag="q", bufs=2)
                    nc.tensor.matmul(q_ps[:], ones64[:], p_sb[:], start=True, stop=True)
                    m_ps = stps.tile([K, 512], F32, tag="m", bufs=2)
                    nc.tensor.matmul(m_ps[:], ssr_sb[:], lut_sb[:, cols],
                                     start=True, stop=True)

                    ms_sb = stp.tile([K, 512], F32, tag="ms", bufs=2)
                    nc.scalar.mul(ms_sb[:], m_ps[:], 1.0 / np.sqrt(B))
                    t1_sb = stp.tile([K, 512], F32, tag="t1", bufs=2)
                    nc.vector.tensor_tensor(t1_sb[:], ms_sb[:], ms_sb[:], op=ALU.mult)
                    v_sb = stp.tile([K, 512], F32, tag="v", bufs=2)
                    nc.vector.tensor_tensor(v_sb[:], q_ps[:], t1_sb[:], op=ALU.subtract)
                    sd_sb = stp.tile([K, 512], F32, tag="sd", bufs=2)
                    nc.scalar.activation(sd_sb[:], v_sb[:], AF.Sqrt,
                                         bias=eps_col[:], scale=1.0 / (B - 1))
                    inv_sb = stp.tile([K, 512], F32, tag="inv", bufs=2)
                    nc.vector.reciprocal(inv_sb[:], sd_sb[:])

                    grep_ps = stps.tile([K, 512], F32, tag="grep", bufs=1)
                    nc.tensor.matmul(grep_ps[:], ones64[0:1, :], gam_sb[:, cols],
                                     start=True, stop=True)
                    sc_sb = stp.tile([K, 512], F32, tag="sc", bufs=2)
                    nc.vector.tensor_tensor(sc_sb[:], inv_sb[:], grep_ps[:], op=ALU.mult)
                    nc.vector.tensor_tensor(lut_aug[0:K, cols], lut_sb[:, cols],
                                            sc_sb[:], op=ALU.mult)
                    msn_sb = stp.tile([K, 512], F32, tag="msn", bufs=2)
                    nc.scalar.mul(msn_sb[:], m_ps[:], -1.0 / B)
                    sh_sb = stp.tile([K, 512], F32, tag="sh", bufs=2)
                    nc.vector.tensor_tensor(sh_sb[:], msn_sb[:], sc_sb[:], op=ALU.mult)
                    nc.scalar.copy(lut_aug[K:K + 1, cols], sh_sb[0:1, :])

            # ---------------- pass 2 -----------------
            with (
                tc.tile_pool(name="x2p", bufs=2) as x2p,
                tc.tile_pool(name="op", bufs=2) as op_pool,
                tc.tile_pool(name="l2p", bufs=2) as l2p,
                tc.tile_pool(name="yps", bufs=3, space="PSUM") as yps,
            ):
                for i in range(NCH):
                    rows = bass.ts(i, CH)
                    s2aT_ps = yps.tile([K + 1, 128], F32, tag="saT", bufs=2)
                    nc.tensor.transpose(
                        s2aT_ps[:], s2aug[:, i * (K + 1):(i + 1) * (K + 1)], ident[:]
                    )
                    s2aT_sb = l2p.tile([K + 1, 128], F32)
                    nc.scalar.copy(s2aT_sb[:], s2aT_ps[:])

                    x2_i = x2p.tile([128, D], F32)
                    nc.sync.dma_start(x2_i[:], x_in[rows, :])
                    o_sb = op_pool.tile([128, D], F32)
                    for sl in range(NS):
                        cols = bass.ts(sl, 512)
                        y_ps = yps.tile([128, 512], F32, tag="y")
                        nc.tensor.matmul(y_ps[:], s2aT_sb[:], lut_aug[:, cols],
                                         start=True, stop=True)
                        nc.vector.tensor_tensor(o_sb[:, cols], y_ps[:], x2_i[:, cols],
                                                op=ALU.add)
                    nc.sync.dma_start(out_ext[rows, :], o_sb[:])

    _split_sync_waits(nc)
    return nc


_CACHED_NC = None


def _get_nc():
    global _CACHED_NC
    if _CACHED_NC is None:
        _CACHED_NC = build_nc()
    return _CACHED_NC


def _make_in_maps(inputs):
    x = np.ascontiguousarray(np.asarray(inputs["x"], dtype=np.float32))
    hyp = np.ascontiguousarray(
        np.asarray(inputs["hypernet_output"], dtype=np.float32).reshape(B, K * K)
    )
    rdt = np.ascontiguousarray(np.asarray(inputs["down_proj_right"], np.float32).T)
    ldt = np.ascontiguousarray(np.asarray(inputs["down_proj_left"], np.float32).T)
    rut = np.ascontiguousarray(np.asarray(inputs["up_proj_right"], np.float32).T)
    lut = np.ascontiguousarray(np.asarray(inputs["up_proj_left"], np.float32).T)
    gamma = np.asarray(inputs["gamma"], np.float32).reshape(1, D)
    bias1 = np.asarray(inputs["bias1"], np.float32).reshape(BOT)
    # bias2 cancels exactly under batchnorm (shift invariance) — unused.

    in_maps = []
    for c in range(N_CORES):
        sl = slice(c * BC, (c + 1) * BC)
        in_maps.append({
            "x": x[sl], "hyp": hyp[sl],
            "rdt": rdt, "ldt": ldt, "rut": rut, "lut": lut,
            "gamma": gamma, "bias1": bias1,
        })
    return in_maps


def _run(inputs, trace=False):
    if trace:
        _install_ntff_hook_shim()
        _patch_upload_artifacts()
    nc = _get_nc()
    in_maps = _make_in_maps(inputs)
    res = run_bass_kernel_spmd(
        nc, in_maps, core_ids=list(range(N_CORES)), trace=trace
    )
    out = np.concatenate([res.results[i]["out"] for i in range(N_CORES)], axis=0)
    return out, res


def kernel(**inputs) -> np.ndarray:
    out, _ = _run(inputs, trace=False)
    return out


def kernel_traced(**inputs):
    """Returns (out, exec_time_ns) — used by test.py for profiling."""
    out, res = _run(inputs, trace=True)
    return out, res.exec_time_ns
